# revision 1
# baseline (speedup 1.0000x reference)
"""Trainium2 Bass kernel for nn_DenseBlockEnd (gnn_message_passing).

Computes, for each graph b (B=512, MAX_ATOM=256, F=256):
    out[b] = relu(mask[b] * (node[b] + sum_l beta1*A_l[b] @ W_in[l]
                                     + beta2*BO[b] @ W_out[0]))
with mask[b, m] = (m < mol_slice[b]).

Strategy: data-parallel over the batch axis, 64 graphs per NeuronCore on 8
cores.  The three layer matmuls are fused into a single K=768 accumulation
against host-premultiplied (beta * W) weight chunks.  Activations are cast
f32->bf16 during the HBM->SBUF DMA, transposed on the TensorEngine (so the
contraction dim lands on partitions), then matmul'd in bf16 with f32 PSUM
accumulation.  node_features are added in f32 on the VectorEngine and the
row mask + relu are applied on the ScalarEngine via a per-partition scale.
"""

import numpy as np
import ml_dtypes
from contextlib import ExitStack

import concourse.bass as bass
import concourse.tile as tile
from concourse import bacc, mybir
from concourse import bass_utils

B, M, F = 512, 256, 256
NCORES = 8
BS = B // NCORES          # graphs per core
G = 4                     # graphs per pipeline batch
NB = BS // G              # pipeline batches
NSLAB = 3                 # inblock_acts[0], inblock_acts[1], block_outputs[0]
P = 128

F32 = mybir.dt.float32
BF16 = mybir.dt.bfloat16

_cached_nc = None


def _build_nc():
    nc = bacc.Bacc(trn_type="TRN2", target_bir_lowering=False, debug=False)

    node_d = nc.dram_tensor("node", [BS, M, F], F32, kind="ExternalInput").ap()
    a0_d = nc.dram_tensor("a0", [BS, M, F], F32, kind="ExternalInput").ap()
    a1_d = nc.dram_tensor("a1", [BS, M, F], F32, kind="ExternalInput").ap()
    bo_d = nc.dram_tensor("bo", [BS, M, F], F32, kind="ExternalInput").ap()
    wc_d = nc.dram_tensor("wc", [2 * NSLAB, P, F], BF16, kind="ExternalInput").ap()
    mask_d = nc.dram_tensor("maskt", [2, P, BS], F32, kind="ExternalInput").ap()
    ident_d = nc.dram_tensor("ident", [P, P], BF16, kind="ExternalInput").ap()
    out_d = nc.dram_tensor("out", [BS, M, F], F32, kind="ExternalOutput").ap()

    slabs_d = (a0_d, a1_d, bo_d)

    with tile.TileContext(nc) as tc, ExitStack() as ctx:
        const_pool = ctx.enter_context(tc.tile_pool(name="const", bufs=1))
        raw_pool = ctx.enter_context(tc.tile_pool(name="raw", bufs=4))
        at_pool = ctx.enter_context(tc.tile_pool(name="at", bufs=30))
        out_pool = ctx.enter_context(tc.tile_pool(name="outp", bufs=3))
        psum_t_pool = ctx.enter_context(
            tc.tile_pool(name="psum_t", bufs=3, space="PSUM")
        )
        psum_o_pool = ctx.enter_context(
            tc.tile_pool(name="psum_o", bufs=5, space="PSUM")
        )

        # Constants: combined weights [f_chunk, o], row masks, identity.
        w_sb = const_pool.tile([P, 2 * NSLAB, F], BF16, name="w_sb")
        nc.sync.dma_start(w_sb[:], wc_d.rearrange("c p o -> p c o"))
        mask_sb = const_pool.tile([P, 2, BS], F32, name="mask_sb")
        nc.sync.dma_start(mask_sb[:], mask_d.rearrange("t p g -> p t g"))
        ident_sb = const_pool.tile([P, P], BF16, name="ident_sb")
        nc.sync.dma_start(ident_sb[:], ident_d[:])

        # Atom rows are packed two-per-partition (m = 2p + j, j inner) so every
        # DMA descriptor covers 2 contiguous DRAM rows (2 KB) instead of 1.
        evac_parity = 0
        for bi in range(NB):
            g0 = bi * G
            # ---- loads ----
            node_raw = raw_pool.tile([P, G, 2, F], F32, name="node_raw", tag="node")
            nc.sync.dma_start(
                node_raw[:],
                node_d[g0 : g0 + G].rearrange("g (p j) f -> p g j f", j=2),
            )
            a_raws = []
            for s in range(NSLAB):
                a_raw = raw_pool.tile(
                    [P, G, 2, F], BF16, name=f"a{s}_raw", tag=f"a{s}"
                )
                # SWDGE DMA with f32 -> bf16 cast in flight.  The first batch
                # loads per-graph so the PE pipeline starts ASAP.
                if bi == 0:
                    for gi in range(G):
                        nc.gpsimd.dma_start(
                            a_raw[:, gi : gi + 1],
                            slabs_d[s][g0 + gi : g0 + gi + 1].rearrange(
                                "g (p j) f -> p g j f", j=2
                            ),
                        )
                else:
                    nc.gpsimd.dma_start(
                        a_raw[:],
                        slabs_d[s][g0 : g0 + G].rearrange("g (p j) f -> p g j f", j=2),
                    )
                a_raws.append(a_raw)

            out_sb = out_pool.tile([P, G, 2, F], F32, name="out_sb", tag="out")

            for gi in range(G):
                # ---- transpose A slabs: [m, f] -> [f, m] via PE ----
                ats = []
                for s in range(NSLAB):
                    psum_t = psum_t_pool.tile(
                        [P, 2, F], BF16, name=f"psum_t{s}", tag="pt"
                    )
                    for j in range(2):
                        for fc in range(2):
                            nc.tensor.transpose(
                                psum_t[:, fc, j * P : (j + 1) * P],
                                a_raws[s][:, gi, j, fc * P : (fc + 1) * P],
                                ident_sb[:],
                            )
                    at = at_pool.tile([P, 2, F], BF16, name=f"at{s}", tag="at")
                    nc.vector.tensor_copy(at[:], psum_t[:])
                    ats.append(at)

                # ---- matmuls: psum_o[m, o] = sum_s,fc A_s^T(fc, m)^T @ W(s, fc) ----
                psum_o = psum_o_pool.tile([P, 2, F], F32, name="psum_o", tag="po")
                for j in range(2):
                    first = True
                    for s in range(NSLAB):
                        for fc in range(2):
                            nc.tensor.matmul(
                                psum_o[:, j, :],
                                ats[s][:, fc, j * P : (j + 1) * P],
                                w_sb[:, 2 * s + fc, :],
                                start=first,
                                stop=(s == NSLAB - 1 and fc == 1),
                            )
                            first = False

                # ---- epilogue: add node (f32), then relu(mask * x) ----
                for j in range(2):
                    nc.vector.tensor_add(
                        out_sb[:, gi, j, :],
                        psum_o[:, j, :],
                        node_raw[:, gi, j, :],
                    )
                    nc.scalar.activation(
                        out_sb[:, gi, j, :],
                        out_sb[:, gi, j, :],
                        mybir.ActivationFunctionType.Relu,
                        scale=mask_sb[:, j, g0 + gi : g0 + gi + 1],
                    )

                # Store per graph so the tail drains as soon as each graph is done.
                nc.scalar.dma_start(
                    out_d[g0 + gi : g0 + gi + 1].rearrange(
                        "g (p j) f -> p g j f", j=2
                    ),
                    out_sb[:, gi : gi + 1],
                )

    nc.compile()
    return nc


def _in_maps(node, inb, bo, wc, maskt_all, ident):
    maps = []
    for c in range(NCORES):
        sl = slice(c * BS, (c + 1) * BS)
        maps.append(
            {
                "node": np.ascontiguousarray(node[sl]),
                "a0": np.ascontiguousarray(inb[0, sl]),
                "a1": np.ascontiguousarray(inb[1, sl]),
                "bo": np.ascontiguousarray(bo[0, sl]),
                "wc": wc,
                "maskt": maskt_all[c],
                "ident": ident,
            }
        )
    return maps


def _prep_in_maps(
    node_features,
    inblock_acts,
    block_outputs,
    mol_slice,
    W_in,
    W_out,
    beta1,
    beta2,
):
    node = np.asarray(node_features, dtype=np.float32)
    inb = np.asarray(inblock_acts, dtype=np.float32)
    bo = np.asarray(block_outputs, dtype=np.float32)
    mol = np.asarray(mol_slice, dtype=np.int32)
    w_in = np.asarray(W_in, dtype=np.float32)
    w_out = np.asarray(W_out, dtype=np.float32)
    b1 = float(np.asarray(beta1).reshape(-1)[0])
    b2 = float(np.asarray(beta2).reshape(-1)[0])

    wc = (
        np.concatenate([b1 * w_in[0], b1 * w_in[1], b2 * w_out[0]], axis=0)
        .reshape(2 * NSLAB, P, F)
        .astype(ml_dtypes.bfloat16)
    )
    mask = (np.arange(M)[None, :] < mol[:, None]).astype(np.float32)  # [B, M]
    # maskt[j, p, g] = mask[g, 2p + j] (row-pair packing, j inner)
    maskt_all = [
        np.ascontiguousarray(
            mask[c * BS : (c + 1) * BS].reshape(BS, P, 2).transpose(2, 1, 0)
        )
        for c in range(NCORES)
    ]
    ident = np.eye(P, dtype=ml_dtypes.bfloat16)
    return _in_maps(node, inb, bo, wc, maskt_all, ident)


def get_nc():
    global _cached_nc
    if _cached_nc is None:
        _cached_nc = _build_nc()
    return _cached_nc


def kernel(**inputs):
    nc = get_nc()
    res = bass_utils.run_bass_kernel_spmd(
        nc, _prep_in_maps(**inputs), core_ids=list(range(NCORES))
    )
    return np.concatenate([res.results[c]["out"] for c in range(NCORES)], axis=0)



# revision 3
# speedup vs baseline: 3.2913x; 3.2913x over previous
"""Trainium2 Bass kernel for nn_DenseBlockEnd (gnn_message_passing).

Computes, for each graph b (B=512, MAX_ATOM=256, F=256):
    out[b] = relu(mask[b] * (node[b] + sum_l beta1*A_l[b] @ W_in[l]
                                     + beta2*BO[b] @ W_out[0]))
with mask[b, m] = (m < mol_slice[b]).

Strategy: the computation is purely row-wise (every valid atom row goes
through the same 3-slab matmul + node add + relu; masked rows are exactly
zero in the output).  So the host compacts the ~N = sum(mol_slice) valid
rows out of B*M = 131072, transposes them to [F, rows] layout, casts to
bf16, and splits them evenly across the 8 cores.  The device then runs a
dense, mask-free pipeline: for each 512-row tile, 12 accumulating matmuls
(stationary 128x128 W chunks, moving 512-wide activations) produce
psum[o, m]; the node term is added on the DVE and relu applied on the
scalar engine; the bf16 result streams back.  The host scatters valid
rows into a zero f32 array.  All tensors move in bf16 (rel err ~3e-3),
halving HBM traffic again on top of the ~2x from row compaction.
"""

import numpy as np
import ml_dtypes
from contextlib import ExitStack

import concourse.bass as bass
import concourse.tile as tile
from concourse import bacc, mybir
from concourse import bass_utils

B, M, F = 512, 256, 256
NCORES = 8
NSLAB = 3                 # inblock_acts[0], inblock_acts[1], block_outputs[0]
P = 128
TILE = 512                # atom rows per pipeline tile

F32 = mybir.dt.float32
BF16 = mybir.dt.bfloat16
BF16_NP = ml_dtypes.bfloat16

_nc_cache = {}


def _build_nc(T):
    nc = bacc.Bacc(trn_type="TRN2", target_bir_lowering=False, debug=False)

    a0_d = nc.dram_tensor("a0", [T, P, 2, TILE], BF16, kind="ExternalInput").ap()
    a1_d = nc.dram_tensor("a1", [T, P, 2, TILE], BF16, kind="ExternalInput").ap()
    bo_d = nc.dram_tensor("bo", [T, P, 2, TILE], BF16, kind="ExternalInput").ap()
    nd_d = nc.dram_tensor("nd", [T, P, 2, TILE], BF16, kind="ExternalInput").ap()
    wc_d = nc.dram_tensor("wc", [2 * NSLAB, P, F], BF16, kind="ExternalInput").ap()
    out_d = nc.dram_tensor("out", [T, P, 2, TILE], BF16, kind="ExternalOutput").ap()

    slabs_d = (a0_d, a1_d, bo_d)

    with tile.TileContext(nc) as tc, ExitStack() as ctx:
        const_pool = ctx.enter_context(tc.tile_pool(name="const", bufs=1))
        in_pool = ctx.enter_context(tc.tile_pool(name="inp", bufs=3))
        out_pool = ctx.enter_context(tc.tile_pool(name="outp", bufs=3))
        psum_pool = ctx.enter_context(tc.tile_pool(name="psum", bufs=4, space="PSUM"))

        # Weights: w_sb[p, s*2+fc, o] = (beta*W)[fc*128+p, o] for slab s.
        w_sb = const_pool.tile([P, 2 * NSLAB, F], BF16, name="w_sb")
        nc.sync.dma_start(w_sb[:], wc_d.rearrange("c p o -> p c o"))

        in_q = (nc.sync, nc.sync, nc.gpsimd, nc.gpsimd)

        for t in range(T):
            ats = []
            for s in range(NSLAB):
                at = in_pool.tile([P, 2, TILE], BF16, name=f"at{s}", tag=f"at{s}")
                in_q[s].dma_start(at[:], slabs_d[s][t])
                ats.append(at)
            nd = in_pool.tile([P, 2, TILE], BF16, name="nd", tag="nd")
            in_q[3].dma_start(nd[:], nd_d[t])

            out_sb = out_pool.tile([P, 2, TILE], BF16, name="out_sb", tag="out")
            for oc in range(2):
                psum = psum_pool.tile([P, TILE], F32, name="psum", tag="ps")
                k = 0
                for s in range(NSLAB):
                    for fc in range(2):
                        nc.tensor.matmul(
                            psum[:],
                            w_sb[:, 2 * s + fc, oc * P : (oc + 1) * P],
                            ats[s][:, fc, :],
                            start=(k == 0),
                            stop=(k == 2 * NSLAB - 1),
                        )
                        k += 1
                nc.vector.tensor_add(out_sb[:, oc, :], psum[:], nd[:, oc, :])
                nc.scalar.activation(
                    out_sb[:, oc, :],
                    out_sb[:, oc, :],
                    mybir.ActivationFunctionType.Relu,
                )
            nc.scalar.dma_start(out_d[t], out_sb[:])

    nc.compile()
    return nc


def get_nc(T=None):
    if T is None:
        T = _last_plan["T"]
    if T not in _nc_cache:
        _nc_cache[T] = _build_nc(T)
    return _nc_cache[T]


_last_plan = None


def _make_plan(mol):
    mask = np.arange(M)[None, :] < mol[:, None]          # [B, M]
    rows_index = np.flatnonzero(mask.ravel())            # valid b*M + m, ordered
    N = rows_index.size
    R = -(-N // (NCORES * TILE)) * TILE                  # rows per core
    return {"rows_index": rows_index, "N": N, "R": R, "T": R // TILE}


def _pack(flat_f32, plan):
    """[B*M, F] f32 -> [NCORES, T, P, 2, TILE] bf16 in transposed layout."""
    rows_index, N, R, T = plan["rows_index"], plan["N"], plan["R"], plan["T"]
    g = np.zeros((NCORES * R, F), dtype=BF16_NP)
    g[:N] = flat_f32[rows_index]
    # g[c, t*TILE+m, fc*P+p] -> out[c, t, p, fc, m]
    g = g.reshape(NCORES, T, TILE, 2, P).transpose(0, 1, 4, 3, 2)
    return np.ascontiguousarray(g)


def _prep_in_maps(
    node_features,
    inblock_acts,
    block_outputs,
    mol_slice,
    W_in,
    W_out,
    beta1,
    beta2,
):
    global _last_plan
    mol = np.asarray(mol_slice, dtype=np.int32)
    plan = _make_plan(mol)
    _last_plan = plan

    node = np.asarray(node_features, dtype=np.float32).reshape(B * M, F)
    inb = np.asarray(inblock_acts, dtype=np.float32)
    bo = np.asarray(block_outputs, dtype=np.float32)
    b1 = float(np.asarray(beta1).reshape(-1)[0])
    b2 = float(np.asarray(beta2).reshape(-1)[0])
    w_in = np.asarray(W_in, dtype=np.float32)
    w_out = np.asarray(W_out, dtype=np.float32)

    wc = (
        np.stack([b1 * w_in[0], b1 * w_in[1], b2 * w_out[0]])
        .reshape(2 * NSLAB, P, F)
        .astype(BF16_NP)
    )

    a0p = _pack(inb[0].reshape(B * M, F), plan)
    a1p = _pack(inb[1].reshape(B * M, F), plan)
    bop = _pack(bo[0].reshape(B * M, F), plan)
    ndp = _pack(node, plan)

    maps = []
    for c in range(NCORES):
        maps.append(
            {
                "a0": a0p[c],
                "a1": a1p[c],
                "bo": bop[c],
                "nd": ndp[c],
                "wc": wc,
            }
        )
    return maps


def _unpack(results, plan):
    rows_index, N, R, T = plan["rows_index"], plan["N"], plan["R"], plan["T"]
    dev = np.stack([results[c]["out"] for c in range(NCORES)])  # [NC,T,P,2,TILE] bf16
    rows = dev.transpose(0, 1, 4, 3, 2).reshape(NCORES * R, F)
    full = np.zeros((B * M, F), dtype=np.float32)
    full[rows_index] = rows[:N].astype(np.float32)
    return full.reshape(B, M, F)


def kernel(**inputs):
    maps = _prep_in_maps(**inputs)
    plan = _last_plan
    nc = get_nc(plan["T"])
    res = bass_utils.run_bass_kernel_spmd(nc, maps, core_ids=list(range(NCORES)))
    return _unpack(res.results, plan)
